# revision 23
# baseline (speedup 1.0000x reference)
"""BiLevelRoutingAttention (spiking) Trainium2 kernel — v2.

Sharding: one (t, b) pair per core (T=4 x B=2 = 8 cores). Cross-core data is
only the routing region sum, via a [128,64] AllReduce among the 4 cores
sharing each b (replica groups {0..3}, {4..7}; core_id = b*4 + t).

v2 structure (vs baseline):
  - kv matmuls + spikes + per-window Grams run FIRST; q matmuls run after,
    overlapping the combine stage.
  - q is computed as q' = Sign(pre - th) in {-1,+1} on the Scalar engine
    (= 2q-1 of the true spike). With column sums CS[he] = sum_hd kv_r[hd,he]
    and S[he] = sum_hd dexp[hd,he] (from a tiny-matmul sweep vs a ones
    vector):  att = (att' + CS)/2,  den = (den' + S)/2, so
       out = att/(den+1e-6) = (att'+CS) * recip(den' + S + 2e-6).
  - Combine transposes G via SBUF->SBUF DMAs (no DRAM round trips): gsb
    [p,(w,c,e)] -> grow4 [(jq,w'),(p32,c,e)], combine matmul 4x-packed with
    tile_position, masked evac (mask64 folds the block-diag mask), then one
    DMA back to kvread [p,(w,c,e)].
  - Engine placement: Scalar = G evac + q Sign + proj bias evac;
    Vector/GpSimd = region sums, kv spikes, combine evac, epilogue
    (den assembly + (att'+CS)*rden via scalar_tensor_tensor; reciprocal
    on vector).
  - PSUM: mm512(3, shared kv/gram/q/proj) + colp(1) + combp(2) + attnp(2)
    = 8 banks.
"""

import numpy as np
import ml_dtypes

T, B, Lt, Lh, Lw, C = 4, 2, 8, 32, 32, 256
WT, WH, WW = 2, 4, 4
LT, LH, LW = Lt // WT, Lh // WH, Lw // WW  # 4, 8, 8
W = WT * WH * WW        # 32 windows
S = LT * LH * LW        # 256 tokens per window
NTOK = W * S            # 8192
H, D = 8, 32
TOPK = 4
NCORES = 8
GROUPS = [[0, 1, 2, 3], [4, 5, 6, 7]]
CCH = 344               # combine chunk (24 chunks over 8256 flat cols)
QROW = 8256             # 32 p32 * 258 (c,e) flat cols per quarter
BF16 = ml_dtypes.bfloat16

_CACHE = {}


def build_kernel(dbg=False):
    from concourse import bacc
    import concourse.mybir as mybir
    import concourse.tile as tile
    from concourse.tile_rust import add_dep_helper
    from concourse.masks import make_identity

    bf = mybir.dt.bfloat16
    f32 = mybir.dt.float32

    nc = bacc.Bacc("TRN2", target_bir_lowering=False, debug=False,
                   num_devices=NCORES)
    if dbg:
        dbg_q = nc.dram_tensor("dbg_q", [128, 2, NTOK], bf,
                               kind="ExternalOutput")
        dbg_kv = nc.dram_tensor("dbg_kv", [128, 32, 258], bf,
                                kind="ExternalOutput")
        dbg_css = nc.dram_tensor("dbg_css", [128, 128], f32,
                                 kind="ExternalOutput")
        dbg_attn = nc.dram_tensor("dbg_attn", [128, 2, NTOK], bf,
                                  kind="ExternalOutput")

    xT = nc.dram_tensor("xT", [2, 128, NTOK], bf, kind="ExternalInput")
    wq = nc.dram_tensor("wq", [128, 2, 2, 128], bf, kind="ExternalInput")
    wkv = nc.dram_tensor("wkv", [128, 2, 512], bf, kind="ExternalInput")
    thq = nc.dram_tensor("thq", [128, 2], f32, kind="ExternalInput")
    thkv = nc.dram_tensor("thkv", [128, 512], f32, kind="ExternalInput")
    wproj = nc.dram_tensor("wproj", [128, 2, 2, 128], bf, kind="ExternalInput")
    bproj = nc.dram_tensor("bproj", [128, 2], f32, kind="ExternalInput")
    bmask = nc.dram_tensor("bmask", [128, 129], bf, kind="ExternalInput")
    mask64 = nc.dram_tensor("mask64", [128, QROW], bf, kind="ExternalInput")
    outT = nc.dram_tensor("outT", [2, 128, NTOK], bf, kind="ExternalOutput")

    cc_in = nc.dram_tensor("cc_in", [128, 64], f32)
    cc_out = nc.dram_tensor("cc_out", [128, 64], f32)

    with tile.TileContext(nc) as tc:
        with (
            tc.tile_pool(name="big", bufs=1) as big_pool,
            tc.tile_pool(name="persist", bufs=1) as pp,
            tc.tile_pool(name="kvs", bufs=6) as kv_pool,
            tc.tile_pool(name="small", bufs=2) as sm_pool,
            tc.tile_pool(name="attn_sb", bufs=3) as asb_pool,
            tc.tile_pool(name="outp", bufs=3) as out_pool,
            tc.tile_pool(name="mm512", bufs=3, space="PSUM") as mm512,
            tc.tile_pool(name="dram", bufs=1, space="DRAM") as dram_pool,
            tc.tile_pool(name="colp", bufs=1, space="PSUM") as colpp,
            tc.tile_pool(name="combp", bufs=2, space="PSUM") as combp,
            tc.tile_pool(name="attnp", bufs=2, space="PSUM") as attnp,
        ):
            # ---- load x (token-sliced so kv can start early) ----
            xsb = big_pool.tile([128, 2, NTOK], bf, tag="xsb")
            for c in range(2):
                for p in range(4):
                    sl = slice(p * 2048, (p + 1) * 2048)
                    nc.sync.dma_start(xsb[:, c, sl], xT[c, :, sl])

            # ---- weights / constants ----
            wq_sb = pp.tile([128, 2, 2, 128], bf)
            nc.sync.dma_start(wq_sb[:], wq[:])
            wkv_sb = pp.tile([128, 2, 512], bf)
            nc.sync.dma_start(wkv_sb[:], wkv[:])
            thq_sb = pp.tile([128, 2], f32)
            nc.sync.dma_start(thq_sb[:], thq[:])
            thkv_sb = pp.tile([128, 512], f32)
            nc.sync.dma_start(thkv_sb[:], thkv[:])
            wproj_sb = pp.tile([128, 2, 2, 128], bf)
            nc.sync.dma_start(wproj_sb[:], wproj[:])
            bproj_sb = pp.tile([128, 2], f32)
            nc.sync.dma_start(bproj_sb[:], bproj[:])
            bmask_sb = pp.tile([128, 129], bf)
            nc.sync.dma_start(bmask_sb[:], bmask[:])
            mask64_sb = pp.tile([128, QROW], bf)
            nc.sync.dma_start(mask64_sb[:], mask64[:])
            id32 = pp.tile([32, 32], f32)
            make_identity(nc, id32[:])
            ones_sb = pp.tile([128, 1], bf)
            nc.vector.memset(ones_sb[:], 1.0)

            # ---- region partial sums -> collective ----
            # region sums via tensor_scalar accum_out (4x DVE mode on bf16)
            region = sm_pool.tile([128, 2, 32], f32, tag="region")
            rscratch = sm_pool.tile([128, 256], bf, tag="rscratch")
            for c in range(2):
                for w in range(32):
                    wsl = slice(w * 256, (w + 1) * 256)
                    nc.vector.tensor_scalar(
                        rscratch[:], xsb[:, c, wsl], 1.0, 0.0,
                        op0=mybir.AluOpType.mult, op1=mybir.AluOpType.add,
                        accum_out=region[:, c, w:w + 1])
            st = nc.sync.dma_start(cc_in[:], region[:].rearrange("p a w -> p (a w)"))
            cc = nc.gpsimd.collective_compute(
                "AllReduce", mybir.AluOpType.add, replica_groups=GROUPS,
                ins=[cc_in[:]], outs=[cc_out[:]],
            )
            add_dep_helper(cc.ins, st.ins, reason="region stored before collective")
            xs_sb = sm_pool.tile([128, 2, 32], f32, tag="xsum")
            ld = nc.sync.dma_start(xs_sb[:], cc_out[:].rearrange("p (a w) -> p a w", w=32))
            add_dep_helper(ld.ins, cc.ins, reason="collective before readback")

            # ---- kv matmuls + spikes + per-window Grams (kv-first) ----
            gsb = big_pool.tile([128, 32, 2, 129], bf, tag="gsb")
            kvts = {}
            for blk in range(16):
                for tci in range(4):
                    tcg = blk * 4 + tci
                    ksl = slice(tcg * 128, (tcg + 1) * 128)
                    kvp = mm512.tile([128, 512], f32, tag="mm512")
                    for c in range(2):
                        nc.tensor.matmul(kvp[:], xsb[:, c, ksl], wkv_sb[:, c, :],
                                         start=(c == 0), stop=(c == 1))
                    kvt = kv_pool.tile([128, 512], bf, tag="kvt")
                    nc.vector.tensor_tensor(kvt[:], kvp[:], thkv_sb[:],
                                            op=mybir.AluOpType.is_ge)
                    kvts[tcg] = kvt
                for w in (blk * 2, blk * 2 + 1):
                    t0, t1 = kvts[2 * w], kvts[2 * w + 1]
                    for c in range(2):
                        gp = mm512.tile([128, 129], f32, tag="mm512")
                        rsl = slice(256 + c * 128, 256 + (c + 1) * 128)
                        ksl2 = slice(c * 128, (c + 1) * 128)
                        nc.tensor.matmul(gp[:, 0:128], t0[:, ksl2], t0[:, rsl],
                                         start=True, stop=False)
                        nc.tensor.matmul(gp[:, 0:128], t1[:, ksl2], t1[:, rsl],
                                         start=False, stop=True)
                        nc.tensor.matmul(gp[:, 128:129], t0[:, ksl2], ones_sb[:],
                                         start=True, stop=False)
                        nc.tensor.matmul(gp[:, 128:129], t1[:, ksl2], ones_sb[:],
                                         start=False, stop=True)
                        nc.scalar.activation(gsb[:, w, c, :], gp[:],
                                             mybir.ActivationFunctionType.Copy)

            # ---- scores -> top-4 selection matrix sel^T ----
            scp = combp.tile([32, 32], f32, tag="comb")
            for c in range(2):
                nc.tensor.matmul(scp[:], xs_sb[:, c, :], xs_sb[:, c, :],
                                 start=(c == 0), stop=(c == 1))
            shifted = sm_pool.tile([32, 32], f32, tag="shifted")
            nc.vector.tensor_scalar(shifted[:], scp[:], 1e6, None,
                                    op0=mybir.AluOpType.add)
            mx8 = sm_pool.tile([32, 8], f32, tag="mx8")
            nc.vector.max(mx8[:], shifted[:])
            nc.vector.memset(mx8[:, TOPK:], 0.0)
            zapped = sm_pool.tile([32, 32], f32, tag="zapped")
            nc.vector.match_replace(out=zapped[:], in_to_replace=mx8[:],
                                    in_values=shifted[:], imm_value=0.0)
            selb = sm_pool.tile([32, 32], f32, tag="selb")
            nc.vector.tensor_tensor(selb[:], shifted[:], zapped[:],
                                    op=mybir.AluOpType.is_gt)
            selT_ps = combp.tile([32, 32], f32, tag="comb")
            nc.tensor.transpose(selT_ps[:], selb[:], id32[:])
            selT = sm_pool.tile([32, 32], bf, tag="selT")
            nc.vector.tensor_copy(selT[:], selT_ps[:])
            selT4 = pp.tile([128, 32], bf)
            for j in range(4):
                nc.sync.dma_start(selT4[32 * j:32 * (j + 1), :], selT[:])

            # ---- q matmuls + Sign (overlaps combine) ----
            qsb = big_pool.tile([128, 2, NTOK], bf, tag="qsb")
            for blk in range(16):
                tsl = slice(blk * 512, (blk + 1) * 512)
                for qc in range(2):
                    qp = mm512.tile([128, 512], f32, tag="mm512")
                    for c in range(2):
                        nc.tensor.matmul(qp[:], wq_sb[:, c, qc, :], xsb[:, c, tsl],
                                         start=(c == 0), stop=(c == 1))
                    nc.scalar.activation(qsb[:, qc, tsl], qp[:],
                                         mybir.ActivationFunctionType.Sign,
                                         bias=thq_sb[:, qc:qc + 1])
                    if dbg:
                        nc.sync.dma_start(dbg_q[:, qc, tsl], qsb[:, qc, tsl])

            # ---- combine via SBUF round trips ----
            # flatten: grow4[32*jq + w', p32*258 + (c,e)] = gsb[32*jq+p32, w', c, e]
            gram_dram = dram_pool.tile([32, 128, 258], bf)
            nc.sync.dma_start(gram_dram[:].rearrange("w p ce -> p w ce"),
                              gsb[:].rearrange("p w c e -> p w (c e)"))
            grow4 = big_pool.tile([128, QROW], bf, tag="grow4")
            nc.sync.dma_start(
                grow4[:],
                gram_dram[:].rearrange("w (jq p) ce -> jq w (p ce)", jq=4),
            )
            kvout = big_pool.tile([128, QROW], bf, tag="kvout")
            for ch in range(24):
                csl = slice(ch * CCH, (ch + 1) * CCH)
                cp = combp.tile([128, CCH], f32, tag="comb")
                for j in range(4):
                    nc.tensor.matmul(cp[32 * j:32 * (j + 1), :],
                                     selT4[32 * j:32 * (j + 1), :],
                                     grow4[32 * j:32 * (j + 1), csl],
                                     start=True, stop=True,
                                     tile_position=(32 * j, 32 * j))
                nc.vector.tensor_tensor(kvout[:, csl], cp[:], mask64_sb[:, csl],
                                        op=mybir.AluOpType.mult)
            # unflatten: kvread[p, w, c, e] = kvout[32*jq + w, p32*258 + (c,e)]
            kvr_dram = dram_pool.tile([32, 128, 258], bf)
            for jq in range(4):
                psl = slice(32 * jq, 32 * (jq + 1))
                nc.sync.dma_start(
                    kvr_dram[:, psl, :].rearrange("w p ce -> w (p ce)"),
                    kvout[psl, :],
                )
            kvread = big_pool.tile([128, 32, 2, 129], bf, tag="kvread")
            nc.sync.dma_start(kvread[:].rearrange("p w c e -> p w (c e)"),
                              kvr_dram[:].rearrange("w p ce -> p w ce"))
            if dbg:
                nc.sync.dma_start(dbg_kv[:],
                                  kvread[:].rearrange("p w c e -> p w (c e)"))
            # dexp[p, w, c, 0:128] = ksum[p] * bmask[p, e]  (den stationary)
            dexp = big_pool.tile([128, 32, 2, 128], bf, tag="dexp")
            nc.gpsimd.tensor_tensor(
                dexp[:],
                kvread[:, :, :, 128:129].to_broadcast([128, 32, 2, 128]),
                bmask_sb[:, None, None, 0:128].to_broadcast([128, 32, 2, 128]),
                op=mybir.AluOpType.mult,
            )

            # ---- correction sums CS/S (tiny-matmul sweep vs ones) ----
            colp = colpp.tile([128, 128], f32, tag="colp")
            for w in range(32):
                for c in range(2):
                    i = 4 * w + 2 * c
                    nc.tensor.matmul(colp[:, i:i + 1], kvread[:, w, c, 0:128],
                                     ones_sb[:], start=True, stop=True)
                    nc.tensor.matmul(colp[:, i + 1:i + 2], dexp[:, w, c, :],
                                     ones_sb[:], start=True, stop=True)
            # cs_sb[:, i] = CS for (w,c) pair i; s2_sb[:, i] = S + 2e-6
            cs_sb = sm_pool.tile([128, 64], f32, tag="cs")
            nc.vector.tensor_copy(
                cs_sb[:], colp[:].rearrange("p (i two) -> p two i", two=2)[:, 0, :])
            s2_sb = sm_pool.tile([128, 64], f32, tag="s2")
            nc.vector.tensor_scalar(
                s2_sb[:], colp[:].rearrange("p (i two) -> p two i", two=2)[:, 1, :],
                2e-3, None, op0=mybir.AluOpType.add)
            if dbg:
                css_cat = sm_pool.tile([128, 128], f32, tag="csscat")
                nc.vector.tensor_copy(css_cat[:, 0:64], cs_sb[:])
                nc.vector.tensor_copy(css_cat[:, 64:128], s2_sb[:])
                nc.sync.dma_start(dbg_css[:], css_cat[:])

            # ---- attention + epilogue + projection ----
            for blk in range(16):
                attn_nb = asb_pool.tile([128, 2, 512], bf, tag="attnb")
                for wi, w in enumerate((blk * 2, blk * 2 + 1)):
                    wsl = slice(w * 256, (w + 1) * 256)
                    for c in range(2):
                        i = 2 * w + c
                        adp = attnp.tile([128, 512], f32, tag="attn")
                        nc.tensor.matmul(adp[:, 0:256], kvread[:, w, c, 0:128],
                                         qsb[:, c, wsl], start=True, stop=True)
                        nc.tensor.matmul(adp[:, 256:512], dexp[:, w, c, :],
                                         qsb[:, c, wsl], start=True, stop=True)
                        # den_sc = den' + (S + 2e-6)  (scalar engine, from PSUM)
                        den_sc = out_pool.tile([128, 256], f32, tag="densc",
                                               bufs=4)
                        nc.scalar.activation(
                            den_sc[:], adp[:, 256:512],
                            mybir.ActivationFunctionType.Identity,
                            bias=s2_sb[:, i:i + 1])
                        nc.vector.reciprocal_approx_fast(out=den_sc[:],
                                                         in_=den_sc[:])
                        # out = (att' + CS) * rden
                        if w % 2 == 0:
                            nc.vector.scalar_tensor_tensor(
                                attn_nb[:, c, 256 * wi:256 * (wi + 1)],
                                adp[:, 0:256], cs_sb[:, i:i + 1], den_sc[:],
                                op0=mybir.AluOpType.add,
                                op1=mybir.AluOpType.mult)
                        else:
                            att_sb = out_pool.tile([128, 256], f32,
                                                   tag="attsb", bufs=4)
                            nc.scalar.activation(
                                att_sb[:], adp[:, 0:256],
                                mybir.ActivationFunctionType.Identity,
                                bias=cs_sb[:, i:i + 1])
                            nc.gpsimd.tensor_tensor(
                                attn_nb[:, c, 256 * wi:256 * (wi + 1)],
                                att_sb[:], den_sc[:],
                                op=mybir.AluOpType.mult)
                tsl = slice(blk * 512, (blk + 1) * 512)
                if dbg:
                    for c in range(2):
                        nc.sync.dma_start(dbg_attn[:, c, tsl], attn_nb[:, c, :])
                for pc in range(2):
                    pjp = mm512.tile([128, 512], f32, tag="mm512")
                    for ec in range(2):
                        nc.tensor.matmul(pjp[:], wproj_sb[:, ec, pc, :],
                                         attn_nb[:, ec, :],
                                         start=(ec == 0), stop=(ec == 1))
                    osb = out_pool.tile([128, 512], bf, tag="osb")
                    nc.scalar.activation(osb[:], pjp[:],
                                         mybir.ActivationFunctionType.Identity,
                                         bias=bproj_sb[:, pc:pc + 1])
                    nc.sync.dma_start(outT[pc, :, tsl], osb[:])

    nc.compile()
    return nc


def _prep_shared(w_qkv, b_qkv, w_proj, b_proj):
    wq_a = w_qkv[:, 0:256].reshape(2, 128, 2, 128).transpose(1, 0, 2, 3)
    wkv_a = w_qkv[:, 256:768].reshape(2, 128, 512).transpose(1, 0, 2)
    th = 2.0 - b_qkv
    # Sign bias: pre + (-th); Sign(x) >= 0 -> spike
    thq_a = -th[0:256].reshape(2, 128).T
    thkv_a = np.broadcast_to(th[256:768], (128, 512))
    wproj_a = w_proj.reshape(2, 128, 2, 128).transpose(1, 0, 2, 3)
    bproj_a = b_proj.reshape(2, 128).T
    i = np.arange(128)[:, None]
    j = np.arange(129)[None, :]
    bmask_a = ((i // 32) == (j // 32)) | (j == 128)
    # mask64[32*jq + w, p32*258 + c*129 + e] = bmask[32*jq + p32, e]
    row = np.arange(128)[:, None]          # 32*jq + w
    col = np.arange(QROW)[None, :]         # p32*258 + c*129 + e
    jq = row // 32
    p32 = col // 258
    e = (col % 258) % 129
    hd = 32 * jq + p32
    mask64_a = ((hd // 32) == (e // 32)) | (e == 128)
    return {
        "wq": np.ascontiguousarray(wq_a).astype(BF16),
        "wkv": np.ascontiguousarray(wkv_a).astype(BF16),
        "thq": np.ascontiguousarray(thq_a).astype(np.float32),
        "thkv": np.ascontiguousarray(thkv_a).astype(np.float32),
        "wproj": np.ascontiguousarray(wproj_a).astype(BF16),
        "bproj": np.ascontiguousarray(bproj_a).astype(np.float32),
        "bmask": bmask_a.astype(BF16),
        "mask64": mask64_a.astype(BF16),
    }


def window_partition(x):
    """[T,B,Lt,Lh,Lw,C] -> [T,B,NTOK,C] with tokens in (w, s) order."""
    Tb, Bb = x.shape[0], x.shape[1]
    xw = x.reshape(Tb, Bb, WT, LT, WH, LH, WW, LW, C)
    xw = xw.transpose(0, 1, 2, 4, 6, 3, 5, 7, 8)
    return np.ascontiguousarray(xw).reshape(Tb, Bb, NTOK, C)


def window_reverse(o):
    """[NTOK, C] -> [Lt, Lh, Lw, C]."""
    o = o.reshape(WT, WH, WW, LT, LH, LW, C)
    o = o.transpose(0, 3, 1, 4, 2, 5, 6)
    return np.ascontiguousarray(o).reshape(Lt, Lh, Lw, C)


def run_kernel_spmd(nc, in_maps, **kwargs):
    from concourse.bass_utils import run_bass_kernel_spmd
    return run_bass_kernel_spmd(nc, in_maps, core_ids=list(range(NCORES)), **kwargs)


def make_in_maps(x, w_qkv, b_qkv, w_proj, b_proj):
    shared = _prep_shared(w_qkv, b_qkv, w_proj, b_proj)
    xw = window_partition(x)
    in_maps = []
    for core in range(NCORES):
        b, t = core // 4, core % 4
        xt = np.ascontiguousarray(xw[t, b].T).astype(BF16)  # [C, NTOK]
        in_maps.append({**shared, "xT": xt.reshape(2, 128, NTOK)})
    return in_maps


def unpack_out(res):
    out = np.empty((T, B, Lt, Lh, Lw, C), dtype=np.float32)
    for core in range(NCORES):
        b, t = core // 4, core % 4
        oT = np.asarray(res.results[core]["outT"],
                        dtype=np.float32).reshape(256, NTOK)
        out[t, b] = window_reverse(np.ascontiguousarray(oT.T))
    return out


def kernel(x, w_qkv, b_qkv, w_proj, b_proj):
    x = np.asarray(x, dtype=np.float32)
    w_qkv = np.asarray(w_qkv, dtype=np.float32)
    b_qkv = np.asarray(b_qkv, dtype=np.float32)
    w_proj = np.asarray(w_proj, dtype=np.float32)
    b_proj = np.asarray(b_proj, dtype=np.float32)

    if "nc" not in _CACHE:
        _CACHE["nc"] = build_kernel()
    nc = _CACHE["nc"]

    in_maps = make_in_maps(x, w_qkv, b_qkv, w_proj, b_proj)
    res = run_kernel_spmd(nc, in_maps)
    return unpack_out(res)
